# revision 1
# baseline (speedup 1.0000x reference)
"""Trainium2 Bass kernel for nn_ActionScoringModel (LRU + max-pool + tanh MLP).

Strategy: data-parallel over batch (64 = 8 cores x 8 batches). No collectives.
Per core:
  - obs [8,2048,384] f32 staged to SBUF, PE-transposed (f32r) to obsT [d, s]
  - projections u_re/u_im via f32r matmuls with duplicated stationaries
    A=[Bg_re|Bg_im], B=[Bg_im|Bg_re] so the complex phase rotation is
    partition-aligned:  v = cos2 (.) uA + sinpm (.) uB
  - LRU scan  h_t = lam*h_{t-1} + u_t  with lam = rho*e^{i theta} is done as
    two REAL scans on g = e^{-i theta s} h:   g = rho*g_prev + v
    (vector/gpsimd tensor_tensor_scan, fp32 state)
  - output rotation is folded into the C matmul: p1 = cos2 (.) g,
    p2 = sin2 (.) g, y = CM1 @ p1 + CM2 @ p2 + D @ obsT  (PSUM accumulate)
  - latent = max_s y (tensor_reduce), then tanh MLP head on [latent, act]
"""

import sys
import numpy as np
from contextlib import ExitStack

for _p in ("/opt/trn_rl_repo",):
    if _p not in sys.path:
        sys.path.insert(0, _p)

import ml_dtypes
import concourse.bass as bass
import concourse.tile as tile
from concourse import bacc, mybir
from concourse.bass_utils import run_bass_kernel_spmd

BF16 = mybir.dt.bfloat16
F32 = mybir.dt.float32
F32R = mybir.dt.float32r

B_, S_, A_, D_IN, H_, D_OUT, D_MLP = 64, 2048, 128, 384, 64, 64, 64
NCORES = 8
NB = B_ // NCORES          # 8 batches per core
NSB = S_ // 512            # 4 s-blocks of 512
NSJ = S_ // 128            # 16 s-tiles of 128
NDC = D_IN // 128          # 3 d-chunks
NEG_INF = -3.0e38


def _build_nc(trace_tag=""):
    nc = bacc.Bacc("TRN2", target_bir_lowering=False, debug=False,
                   num_devices=1)

    # ---- DRAM I/O ----
    obs_d = nc.dram_tensor("obs", [NB, S_, D_IN], F32, kind="ExternalInput").ap()
    act_d = nc.dram_tensor("act", [NB, A_, D_IN], F32, kind="ExternalInput").ap()
    rho_d = nc.dram_tensor("rho", [128, S_], F32, kind="ExternalInput").ap()
    cosSb_d = nc.dram_tensor("cosSb", [128, S_], BF16, kind="ExternalInput").ap()
    sinpmb_d = nc.dram_tensor("sinpmb", [128, S_], BF16, kind="ExternalInput").ap()
    cos2b_d = nc.dram_tensor("cos2b", [128, S_], BF16, kind="ExternalInput").ap()
    sin2b_d = nc.dram_tensor("sin2b", [128, S_], BF16, kind="ExternalInput").ap()
    statA_d = nc.dram_tensor("statA", [NDC, 128, 128], BF16, kind="ExternalInput").ap()
    perm_d = nc.dram_tensor("perm", [128, 128], BF16, kind="ExternalInput").ap()
    statD_d = nc.dram_tensor("statD", [NDC, 128, D_OUT], BF16, kind="ExternalInput").ap()
    cm1_d = nc.dram_tensor("cm1", [128, D_OUT], BF16, kind="ExternalInput").ap()
    cm2_d = nc.dram_tensor("cm2", [128, D_OUT], BF16, kind="ExternalInput").ap()
    w1lat_d = nc.dram_tensor("w1lat", [64, 64], BF16, kind="ExternalInput").ap()
    w1act_d = nc.dram_tensor("w1act", [NDC, 128, D_MLP], BF16, kind="ExternalInput").ap()
    w2_d = nc.dram_tensor("w2", [64, 32], BF16, kind="ExternalInput").ap()
    w3_d = nc.dram_tensor("w3", [32, 1], BF16, kind="ExternalInput").ap()
    b1_d = nc.dram_tensor("b1", [64, 1], F32, kind="ExternalInput").ap()
    b2_d = nc.dram_tensor("b2", [32, 1], F32, kind="ExternalInput").ap()
    b3_d = nc.dram_tensor("b3", [1, 1], F32, kind="ExternalInput").ap()
    ident_d = nc.dram_tensor("ident", [128, 128], BF16, kind="ExternalInput").ap()
    out_d = nc.dram_tensor("out", [1, NB * A_], F32, kind="ExternalOutput").ap()

    MULT = mybir.AluOpType.mult
    ADD = mybir.AluOpType.add
    MAX = mybir.AluOpType.max
    TANH = mybir.ActivationFunctionType.Tanh

    with tile.TileContext(nc) as tc, ExitStack() as ctx:
        const = ctx.enter_context(tc.tile_pool(name="const", bufs=1))
        stage = ctx.enter_context(tc.tile_pool(name="stage", bufs=2))
        obsT_pool = ctx.enter_context(tc.tile_pool(name="obsT", bufs=3))
        work = ctx.enter_context(tc.tile_pool(name="work", bufs=2))
        ptrans = ctx.enter_context(tc.tile_pool(name="ptrans", bufs=2, space="PSUM"))
        pAB = ctx.enter_context(tc.tile_pool(name="pAB", bufs=2, space="PSUM"))
        pA2 = ctx.enter_context(tc.tile_pool(name="pA2", bufs=2, space="PSUM"))
        pY = ctx.enter_context(tc.tile_pool(name="pY", bufs=2, space="PSUM"))
        small = ctx.enter_context(tc.tile_pool(name="small", bufs=1))

        def load_const(ap_d, shape, dtype, suffix=""):
            nm = f"c_{ap_d.tensor.name}{suffix}"
            t = const.tile(shape, dtype, tag=nm, name=nm)
            nc.sync.dma_start(out=t[:], in_=ap_d)
            return t

        rho = load_const(rho_d, [128, S_], F32)
        cosSb = load_const(cosSb_d, [128, S_], BF16)
        sinpmb = load_const(sinpmb_d, [128, S_], BF16)
        cos2b = load_const(cos2b_d, [128, S_], BF16)
        sin2b = load_const(sin2b_d, [128, S_], BF16)
        ident = load_const(ident_d, [128, 128], BF16)
        statA = [load_const(statA_d[k], [128, 128], BF16, f"{k}") for k in range(NDC)]
        perm = load_const(perm_d, [128, 128], BF16)
        statD = [load_const(statD_d[k], [128, D_OUT], BF16, f"{k}") for k in range(NDC)]
        cm1 = load_const(cm1_d, [128, D_OUT], BF16)
        cm2 = load_const(cm2_d, [128, D_OUT], BF16)
        w1lat = load_const(w1lat_d, [64, 64], BF16)
        w1act = [load_const(w1act_d[k], [128, D_MLP], BF16, f"{k}") for k in range(NDC)]
        w2 = load_const(w2_d, [64, 32], BF16)
        w3 = load_const(w3_d, [32, 1], BF16)
        b1 = load_const(b1_d, [64, 1], F32)
        b2 = load_const(b2_d, [32, 1], F32)
        b3 = load_const(b3_d, [1, 1], F32)

        lat = small.tile([64, NB], F32)          # latent columns (max over s)
        latb = small.tile([64, NB], BF16)

        def transpose_in(stg_list, n_rowblocks, dst_tiles):
            """stg_list: tiles of 4 row-blocks [128, 4*384] -> dst_tiles[k]."""
            n_ev = 0
            for jg in range(0, n_rowblocks, 4):
                jn = min(4, n_rowblocks - jg)
                stage_t = stg_list[jg // 4]
                for k in range(NDC):
                    pt = ptrans.tile([128, 512], BF16, tag="ptrans")
                    for j in range(jn):
                        nc.tensor.transpose(
                            out=pt[:, j * 128:(j + 1) * 128],
                            in_=stage_t[:, j * D_IN + k * 128:j * D_IN + (k + 1) * 128],
                            identity=ident[:],
                        )
                    dst = dst_tiles[k]
                    if n_ev % 3 == 0:
                        nc.vector.tensor_copy(
                            out=dst[:, jg * 128:jg * 128 + jn * 128],
                            in_=pt[:, :jn * 128])
                    else:
                        nc.scalar.copy(
                            out=dst[:, jg * 128:jg * 128 + jn * 128],
                            in_=pt[:, :jn * 128])
                    n_ev += 1

        # ---------------- main loop over local batches ----------------
        for b in range(NB):
            # stage obs rows and transpose to obsT (f32r)
            obsT = [obsT_pool.tile([128, S_], BF16, tag=f"obsT{k}", name=f"obsT{k}")
                    for k in range(NDC)]
            stgs = []
            for jg in range(0, NSJ, 4):
                stj = stage.tile([128, 4 * D_IN], BF16, tag=f"stg{jg//4}",
                                 name=f"stg{jg//4}")
                nc.gpsimd.dma_start(
                    out=stj[:].rearrange("p (j d) -> p j d", d=D_IN),
                    in_=obs_d[b, jg * 128:(jg + 4) * 128].rearrange(
                        "(j p) d -> p j d", p=128))
                stgs.append(stj)
            transpose_in(stgs, NSJ, obsT)

            # projections + rotation-in + scan per s-block
            v = work.tile([128, S_], BF16, tag="v")
            for i in range(NSB):
                sl = slice(i * 512, (i + 1) * 512)
                pa = pA2.tile([128, 512], F32, tag="pA")
                pb = pAB.tile([128, 512], F32, tag="pB")
                for k in range(NDC):
                    nc.tensor.matmul(out=pa[:], lhsT=statA[k][:],
                                     rhs=obsT[k][:, sl], start=(k == 0),
                                     stop=(k == NDC - 1))
                uA = work.tile([128, 512], BF16, tag="uA", bufs=3)
                nc.scalar.copy(out=uA[:], in_=pa[:])
                nc.tensor.matmul(out=pb[:], lhsT=perm[:], rhs=uA[:],
                                 start=True, stop=True)
                uB = work.tile([128, 512], BF16, tag="uB", bufs=3)
                nc.scalar.copy(out=uB[:], in_=pb[:])
                t1 = work.tile([128, 512], BF16, tag="t1", bufs=3)
                t2 = work.tile([128, 512], BF16, tag="t2", bufs=3)
                nc.gpsimd.tensor_tensor(out=t1[:], in0=uA[:], in1=cosSb[:, sl], op=MULT)
                nc.vector.tensor_tensor(out=t2[:], in0=uB[:], in1=sinpmb[:, sl], op=MULT)
                nc.gpsimd.tensor_tensor(out=v[:, sl], in0=t1[:], in1=t2[:], op=ADD)

            g = work.tile([128, S_], BF16, tag="g")
            for i in range(NSB):
                sl = slice(i * 512, (i + 1) * 512)
                init = 0.0 if i == 0 else g[:, i * 512 - 1:i * 512]
                nc.vector.tensor_tensor_scan(out=g[:, sl], data0=rho[:, sl],
                                             data1=v[:, sl], initial=init,
                                             op0=MULT, op1=ADD)

            # rotation-out products
            p1 = work.tile([128, S_], BF16, tag="p1")
            p2 = work.tile([128, S_], BF16, tag="p2")
            nc.vector.tensor_tensor(out=p1[:], in0=g[:], in1=cos2b[:], op=MULT)
            nc.vector.tensor_tensor(out=p2[:], in0=g[:], in1=sin2b[:], op=MULT)

            # y = CM1@p1 + CM2@p2 + D@obsT ; latent = max_s y
            ymax = work.tile([64, NSB], F32, tag="ymax")
            for i in range(NSB):
                sl = slice(i * 512, (i + 1) * 512)
                py = pY.tile([64, 512], F32, tag="pY")
                nc.tensor.matmul(out=py[:], lhsT=cm1[:], rhs=p1[:, sl],
                                 start=True, stop=False)
                nc.tensor.matmul(out=py[:], lhsT=cm2[:], rhs=p2[:, sl],
                                 start=False, stop=False)
                for k in range(NDC):
                    nc.tensor.matmul(out=py[:], lhsT=statD[k][:],
                                     rhs=obsT[k][:, sl], start=False,
                                     stop=(k == NDC - 1))
                nc.vector.tensor_reduce(out=ymax[:, i:i + 1], in_=py[:],
                                        axis=mybir.AxisListType.X, op=MAX)
            nc.vector.tensor_reduce(out=lat[:, b:b + 1], in_=ymax[:],
                                    axis=mybir.AxisListType.X, op=MAX)

        # ---------------- MLP head ----------------
        nc.vector.tensor_copy(out=latb[:], in_=lat[:])

        # latW[m, b] = sum_o W1[m, o] * lat[o, b]  (+b1)
        platW = pY.tile([64, 512], F32, tag="pY")
        nc.tensor.matmul(out=platW[:, :NB], lhsT=w1lat[:], rhs=latb[:],
                         start=True, stop=True)
        latWb = small.tile([64, NB], F32)
        nc.vector.tensor_scalar(out=latWb[:], in0=platW[:, :NB],
                                scalar1=b1[:], scalar2=None,
                                op0=ADD)

        # actT via PE transpose
        actT = [obsT_pool.tile([128, NB * A_], BF16, tag=f"actT{k}", name=f"actT{k}")
                for k in range(NDC)]
        asts = []
        for jg in range(0, NB, 4):
            astj = stage.tile([128, 4 * D_IN], BF16, tag=f"stg{jg//4}",
                              name=f"astg{jg//4}")
            nc.gpsimd.dma_start(
                out=astj[:].rearrange("p (j d) -> p j d", d=D_IN),
                in_=act_d[jg:jg + 4].rearrange("b a d -> a b d"))
            asts.append(astj)
        transpose_in(asts, NB * A_ // 128, actT)

        x1 = small.tile([64, NB * A_], BF16)
        for half in range(2):
            hl = slice(half * 512, (half + 1) * 512)
            px = pY.tile([64, 512], F32, tag="pY")
            for k in range(NDC):
                nc.tensor.matmul(out=px[:], lhsT=w1act[k][:], rhs=actT[k][:, hl],
                                 start=(k == 0), stop=(k == NDC - 1))
            for bb in range(4):
                b_idx = half * 4 + bb
                nc.scalar.activation(
                    out=x1[:, b_idx * A_:(b_idx + 1) * A_],
                    in_=px[:, bb * A_:(bb + 1) * A_],
                    func=TANH, bias=latWb[:, b_idx:b_idx + 1], scale=1.0)

        x2 = small.tile([32, NB * A_], BF16)
        for half in range(2):
            hl = slice(half * 512, (half + 1) * 512)
            px = pY.tile([64, 512], F32, tag="pY")
            nc.tensor.matmul(out=px[:32, :], lhsT=w2[:], rhs=x1[:, hl],
                             start=True, stop=True)
            nc.scalar.activation(out=x2[:, hl], in_=px[:32, :], func=TANH,
                                 bias=b2[:], scale=1.0)

        x3 = small.tile([1, NB * A_], F32)
        for half in range(2):
            hl = slice(half * 512, (half + 1) * 512)
            px = pY.tile([64, 512], F32, tag="pY")
            nc.tensor.matmul(out=px[:1, :], lhsT=w3[:], rhs=x2[:, hl],
                             start=True, stop=True)
            nc.scalar.activation(out=x3[:, hl], in_=px[:1, :], func=TANH,
                                 bias=b3[:], scale=1.0)

        nc.sync.dma_start(out=out_d, in_=x3[:])

    nc.compile()
    return nc


_NC_CACHE = {}


def _get_nc():
    if "nc" not in _NC_CACHE:
        _NC_CACHE["nc"] = _build_nc()
    return _NC_CACHE["nc"]


def _host_tables(nu_log, theta_log, gamma_log, B_re, B_im, C_re, C_im, D,
                 W1, b1, W2, b2, W3, b3):
    f64 = np.float64
    rho_h = np.exp(-np.exp(nu_log.astype(f64)))          # [H]
    theta_h = np.exp(theta_log.astype(f64))              # [H]
    gamma_h = np.exp(gamma_log.astype(f64))              # [H]
    s = np.arange(S_, dtype=f64)
    phase = (theta_h[:, None] * s[None, :]) % (2 * np.pi)   # [H, S]
    cos_t = np.cos(phase)
    sin_t = np.sin(phase)

    def dup(x):  # [H,S] -> [128,S]
        return np.concatenate([x, x], axis=0)

    rho = dup(np.broadcast_to(rho_h[:, None], (H_, S_))).astype(np.float32)
    cosSb = dup(cos_t).astype(ml_dtypes.bfloat16)
    sinpmb = np.concatenate([sin_t, -sin_t], axis=0).astype(ml_dtypes.bfloat16)
    cos2b = dup(cos_t).astype(ml_dtypes.bfloat16)
    sin2b = dup(sin_t).astype(ml_dtypes.bfloat16)

    Bg_re = (B_re.astype(f64) * gamma_h[:, None])        # [H, D_IN]
    Bg_im = (B_im.astype(f64) * gamma_h[:, None])
    statA = np.concatenate([Bg_re.T, Bg_im.T], axis=1)   # [D_IN, 128]
    statA = statA.reshape(NDC, 128, 128).astype(ml_dtypes.bfloat16)
    # permutation: out[m] = in[swap(m)] -> lhsT[k, m] = 1 iff k == m ^ 64
    perm = np.zeros((128, 128), dtype=ml_dtypes.bfloat16)
    for m in range(128):
        perm[m ^ 64, m] = 1
    statD = D.T.reshape(NDC, 128, D_OUT).astype(ml_dtypes.bfloat16)

    cm1 = np.concatenate([C_re.T, -C_im.T], axis=0).astype(ml_dtypes.bfloat16)
    cm2 = np.concatenate([-C_im.T, -C_re.T], axis=0).astype(ml_dtypes.bfloat16)

    w1lat = W1[:, :H_].T.astype(ml_dtypes.bfloat16)      # [64 o, 64 m]
    w1act = W1[:, H_:].T.reshape(NDC, 128, D_MLP).astype(ml_dtypes.bfloat16)
    w2 = W2.T.astype(ml_dtypes.bfloat16)                 # [64, 32]
    w3 = W3.T.astype(ml_dtypes.bfloat16)                 # [32, 1]

    return dict(
        rho=rho, cosSb=cosSb, sinpmb=sinpmb, cos2b=cos2b, sin2b=sin2b,
        statA=statA, perm=perm, statD=statD, cm1=cm1, cm2=cm2,
        w1lat=w1lat, w1act=w1act, w2=w2, w3=w3,
        b1=b1.reshape(64, 1).astype(np.float32),
        b2=b2.reshape(32, 1).astype(np.float32),
        b3=b3.reshape(1, 1).astype(np.float32),
        ident=np.eye(128, dtype=ml_dtypes.bfloat16),
    )


def kernel(observations, actions, nu_log, theta_log, gamma_log,
           B_re, B_im, C_re, C_im, D, W1, b1, W2, b2, W3, b3,
           _trace=False, _tmpdir=None):
    observations = np.asarray(observations, dtype=np.float32)
    actions = np.asarray(actions, dtype=np.float32)
    tables = _host_tables(np.asarray(nu_log), np.asarray(theta_log),
                          np.asarray(gamma_log), np.asarray(B_re),
                          np.asarray(B_im), np.asarray(C_re),
                          np.asarray(C_im), np.asarray(D),
                          np.asarray(W1), np.asarray(b1), np.asarray(W2),
                          np.asarray(b2), np.asarray(W3), np.asarray(b3))
    in_maps = []
    for c in range(NCORES):
        m = dict(tables)
        m["obs"] = np.ascontiguousarray(observations[c * NB:(c + 1) * NB])
        m["act"] = np.ascontiguousarray(actions[c * NB:(c + 1) * NB])
        in_maps.append(m)

    nc = _get_nc()
    res = run_bass_kernel_spmd(nc, in_maps, core_ids=list(range(NCORES)),
                               trace=_trace, tmpdir=_tmpdir)
    outs = []
    for c in range(NCORES):
        outs.append(np.asarray(res.results[c]["out"]).reshape(NB, A_, 1))
    full = np.concatenate(outs, axis=0).astype(np.float32)
    if _trace:
        return full, res
    return full

